# revision 55
# baseline (speedup 1.0000x reference)
"""Trainium2 Bass kernel for nn_Attention_3728031613575.

Multi-head attention, B=4 L=2048 D=1024 H=16 (head dim 64), fp32 reference:
    q/k/v = split_heads(x @ W{q,k,v} + b)        [b,h,l,64]
    scores = q k^T + mask * (-1e5)
    out    = softmax(scores) @ v                 -> [b,l,1024]

Sharding (8 cores): core c handles batch b = c//2 and heads (c%2)*8..+8
(batch x head-group data parallel; QKV weights column-sharded by head).
Attention is fully local per core; no collectives.

Per-core algorithm (layouts chosen so softmax lives on the PSUM partition
dim and no probability transposes are ever needed):
  - X^T built once via PE transposes (fp32r), evacuated on the DVE
    (GPSIMD cannot access PSUM, so all PSUM evacuations live on DVE).
  - Q^T/K^T [head-dims, l] and V [l, head-dims] projections in fp32r;
    Q/K biases folded into the DVE PSUM evacuation as a per-partition
    tensor_scalar_add; V bias as a rank-1 matmul term.
  - mask preprocessed once per core:  M_e = exp(-1e5*(m - rowmin(m)))
    (the rowmin bias provides exact max-subtraction for the mask-dominated
    term; the remaining q.k part is range-safe in fp32), stored bf16,
    reloaded transposed through the DMA xbar on the SP queue.
  - per (head-pair, q-block 512, k-block 128):
      S^T[k,q] = K^T.T @ Q^T        (two K=64 matmuls row-tiled on the PE)
      E = exp(S^T)                  (ACT, bf16, from PSUM)
      P~ = E * M_e^T                (DVE, bf16, 2-kb-wide ops)
      O'^T[d,q] += V^T P~ with a ones-column in V producing the softmax
      denominators as row 64 of O'.
  - postproc: PE-transpose O'^T -> [q, 65] (f32r datapath), reciprocal of
    col 64, tensor_scalar normalize, DMA out.

Scheduling: the QKV projection is split into fine-grained work items
(one transpose pair / one 8-matmul projection chain each) drained from a
pump queue between attention k-blocks, so the PE never executes a long
projection blob while the ACT exp stream starves.  The mask pipeline
(DMA in -> DVE rowmin -> ACT exp -> DMA out -> transposed met loads) is
pumped the same way, with every mask block's pipeline placed inside the
projection window where the ACT engine would otherwise idle; transposed
met tiles load just-in-time at each q-block start, queued ahead of the
pumped mask loads.  Each unit's final PV group and postproc defer into
the next unit's first pump slot so unit boundaries never stall the exp
stream; PV groups trail their QK blocks by 9 k-blocks (15 in the
projection-window unit (0,0), whose met tiles land late), with the tail
groups deferred into the next unit, decoupling the ACT exp stream from
PV bursts on the in-order PE queue — this holds the ACT engine at ~98%
occupancy in steady state.  The projection window itself is co-limited
by the serial DMA resource (fp32 mask loads + X/W traffic).
"""

import sys

sys.path.insert(0, "/opt/trn_rl_repo")

import numpy as np

B, L, D, H, DH = 4, 2048, 1024, 16, 64
NCORES = 8
HPC = 8            # heads per core
NPAIR = HPC // 2   # head pairs per core
QBW = 512          # q block width
NQB = L // QBW     # 4 q blocks
NKB = L // 128     # 16 k blocks
NDB = D // 128     # 8 contraction chunks
MASK_C = -100000.0

_CACHE = {}


def _build():
    import concourse.bass as bass
    from concourse import bacc, mybir
    import concourse.tile as tile
    from concourse.masks import make_identity

    F32 = mybir.dt.float32
    F32R = mybir.dt.float32r
    BF16 = mybir.dt.bfloat16
    AF = mybir.ActivationFunctionType
    ALU = mybir.AluOpType
    AX = mybir.AxisListType

    nc = bacc.Bacc(None, target_bir_lowering=False)

    x_d = nc.dram_tensor("x", [L, D], F32R, kind="ExternalInput")
    mask_d = nc.dram_tensor("mask", [L, L], F32, kind="ExternalInput")
    wq_d = nc.dram_tensor("wq", [D, 512], F32R, kind="ExternalInput")
    wk_d = nc.dram_tensor("wk", [D, 512], F32R, kind="ExternalInput")
    wv_d = nc.dram_tensor("wv", [D, 512], F32R, kind="ExternalInput")
    bq_d = nc.dram_tensor("bq", [1, 512], F32, kind="ExternalInput")
    bk_d = nc.dram_tensor("bk", [1, 512], F32, kind="ExternalInput")
    bv_d = nc.dram_tensor("bv", [1, 512], F32R, kind="ExternalInput")
    out_d = nc.dram_tensor("out", [L, 512], F32, kind="ExternalOutput")

    with tile.TileContext(nc) as tc:
        with tc.tile_pool(name="const", bufs=1) as constp, \
             tc.tile_pool(name="persist", bufs=1) as pers, \
             tc.tile_pool(name="dram", bufs=1, space="DRAM") as dramp, \
             tc.tile_pool(name="met", bufs=4) as metp, \
             tc.tile_pool(name="stage", bufs=1) as stagep, \
             tc.tile_pool(name="epool", bufs=3) as epool, \
             tc.tile_pool(name="oevac", bufs=2) as oevacp, \
             tc.tile_pool(name="rpool", bufs=4) as rpool, \
             tc.tile_pool(name="wpool", bufs=1) as wpool, \
             tc.tile_pool(name="xload", bufs=3) as xload, \
             tc.tile_pool(name="xtpool", bufs=1) as xtpool, \
             tc.tile_pool(name="mload", bufs=2) as mload, \
             tc.tile_pool(name="mtmp", bufs=2) as mtmp, \
             tc.tile_pool(name="spsum", bufs=2, space="PSUM") as spsum, \
             tc.tile_pool(name="opsum", bufs=2, space="PSUM") as opsum, \
             tc.tile_pool(name="scratch", bufs=2, space="PSUM") as scratch:

            # ---- constants
            idf32r = constp.tile([128, 128], F32R, name="idf32r", tag="idf32r")
            make_identity(nc, idf32r)
            ones_r = constp.tile([1, 128], F32R, name="ones_r", tag="ones_r")
            nc.vector.memset(ones_r, 1.0)

            # ---- persistent activations
            QT = pers.tile([128, NPAIR, L], F32R, name="QT", tag="QT")
            KT = pers.tile([128, NPAIR, L], F32R, name="KT", tag="KT")
            V = pers.tile([128, NKB, HPC, DH + 1], BF16, name="V", tag="V")
            nc.vector.memset(V[:, :, :, DH], 1.0)

            me_dram = dramp.tile([L, L], BF16, name="me_dram", tag="me_dram")

            # ---- weight / bias loads (DMA queue order matters: X l0 +
            # np0 weight columns gate the first QK chain; masks for qb0
            # follow; everything else pumps behind)

            xts = {}

            def xt_tile(lb):
                if lb not in xts:
                    xts[lb] = xtpool.tile([128, NDB, QBW], F32R,
                                          name="xt%d" % lb, tag="xt")
                return xts[lb]

            # ---------------- mask pipeline (half-width tiles) -----------
            def emit_mask_load(qb):
                mls = []
                for h in range(2):
                    ml = mload.tile([128, L // 2], F32,
                                    name=f"ml{qb}_{h}", tag="ml")
                    nc.sync.dma_start(
                        out=ml,
                        in_=mask_d[qb * 128:(qb + 1) * 128,
                                   h * (L // 2):(h + 1) * (L // 2)])
                    mls.append(ml)
                return mls

            def emit_mask_exp(qb, mls):
                mm = rpool.tile([128, 2], F32, name=f"mm{qb}", tag="rc")
                for h in range(2):
                    nc.vector.tensor_reduce(mm[:, h:h + 1], mls[h],
                                            axis=AX.X, op=ALU.min)
                mbias = rpool.tile([128, 1], F32, name=f"mb{qb}", tag="rc")
                nc.vector.tensor_reduce(mbias, mm, axis=AX.X, op=ALU.min)
                nc.vector.tensor_scalar_mul(mbias, mbias, -MASK_C)
                for h in range(2):
                    me = mtmp.tile([128, L // 2], BF16,
                                   name=f"me{qb}_{h}", tag="me")
                    nc.scalar.activation(me, mls[h], AF.Exp,
                                         bias=mbias, scale=MASK_C)
                    nc.sync.dma_start(
                        out=me_dram[qb * 128:(qb + 1) * 128,
                                    h * (L // 2):(h + 1) * (L // 2)],
                        in_=me)

            met_tiles = {}

            def emit_met(qb_, c):
                """Transposed M_e tiles for (q block, k-block chunk c)."""
                q0 = qb_ * QBW
                mh = metp.tile([128, 4, QBW], BF16,
                               name=f"met{qb_}_{c}", tag="met")
                met_tiles[(qb_, c)] = mh
                for i in range(4):
                    nc.sync.dma_start_transpose(
                        mh[:, i, :],
                        me_dram[q0:q0 + QBW,
                                (4 * c + i) * 128:(4 * c + i + 1) * 128])

            # ---------------- projection work items ---------------------
            xload_tiles = {}

            def emit_xt_load(lb, sh):
                for s in range(2):
                    xl = xload.tile([128, D], F32R,
                                    name=f"xl{lb}_{sh}_{s}", tag="xl")
                    nc.sync.dma_start(
                        out=xl,
                        in_=x_d[lb * 512 + (sh * 2 + s) * 128:
                                lb * 512 + (sh * 2 + s + 1) * 128, :])
                    xload_tiles[(lb, sh, s)] = xl

            def emit_xt_db(lb, sh, db, evac="dve"):
                """Transpose one 128-wide d-chunk of X rows
                [lb*512+sh*256, +256) into xt; fine-grained so the pump can
                hide the evacuation behind the QK stream."""
                xt = xt_tile(lb)
                xls = [xload_tiles[(lb, sh, s)] for s in range(2)]
                tpt = scratch.tile([128, 256], F32R,
                                   name=f"tpd{lb}_{sh}_{db}", tag="sc")
                for s in range(2):
                    nc.tensor.transpose(
                        tpt[:, s * 128:(s + 1) * 128],
                        xls[s][:, db * 128:(db + 1) * 128],
                        idf32r)
                if db == NDB - 1:
                    for s in range(2):
                        del xload_tiles[(lb, sh, s)]
                dst = xt[:, db, sh * 256:(sh + 1) * 256]
                if evac == "act":
                    # ACT is idle during the head; keep the DVE free
                    nc.scalar.activation(dst, tpt, AF.Copy)
                else:
                    nc.vector.tensor_copy(dst, tpt)

            def emit_xt(lb, sh, evac="dve"):
                if (lb, sh, 0) not in xload_tiles:
                    emit_xt_load(lb, sh)
                for db in range(NDB):
                    emit_xt_db(lb, sh, db, evac)

            def emit_qk_chain(w_sb, c0, bias_t, dst, np_, lb, half=None,
                              evac="dve"):
                """One projection chain into dst (KT slice or QT chunk).
                w_sb holds W columns starting at head-col c0*128.  half
                restricts to a 256-wide l-half (for the head chains that
                must not wait on the second X^T half)."""
                l0, lw = (0, 512) if half is None else (half * 256, 256)
                qp = scratch.tile([128, lw],
                                  F32, name=f"qp{np_}_{lb}_{id(w_sb) % 97}",
                                  tag="sc")
                for db in range(NDB):
                    nc.tensor.matmul(
                        qp,
                        w_sb[:, db, (np_ - c0) * 128:(np_ - c0 + 1) * 128],
                        xt_tile(lb)[:, db, l0:l0 + lw],
                        start=(db == 0), stop=(db == NDB - 1))
                dst_sl = dst[:, np_, lb * 512 + l0:lb * 512 + l0 + lw]
                if evac == "act":
                    nc.scalar.activation(dst_sl, qp, AF.Identity,
                                         bias=bias_t[:, np_:np_ + 1])
                else:
                    nc.vector.tensor_scalar_add(dst_sl, qp,
                                                bias_t[:, np_:np_ + 1])

            def emit_v_chain(kb):
                vp = scratch.tile([128, 512], F32, name=f"vp{kb}", tag="sc")
                for db in range(NDB):
                    nc.tensor.matmul(
                        vp,
                        xt_tile(kb // 4)[:, db, (kb % 4) * 128:
                                         (kb % 4 + 1) * 128],
                        wv[:, db, :],
                        start=(db == 0), stop=(db == NDB - 1))
                # bias folded in via the broadcast tile (built once): V+bv
                nc.vector.tensor_tensor(
                    out=V[:, kb, :, 0:DH],
                    in0=vp.rearrange("p (h d) -> p h d", h=HPC),
                    in1=bvb.rearrange("p (h d) -> p h d", h=HPC),
                    op=ALU.add)

            # ---------------- pump ---------------------------------------
            work = []

            def pump(n):
                for _ in range(n):
                    if work:
                        work.pop(0)()

            # ---------------- attention unit ------------------------------
            def emit_attn_pair(qb_, pr, rate=2, delay=9, met_kb=0,
                               flush_tail=False, defer_stride=1):
                hA, hB = 2 * pr, 2 * pr + 1
                q0 = qb_ * QBW
                oa = opsum.tile([DH + 1, QBW], F32,
                                name=f"oa{qb_}_{pr}", tag="o")
                ob = opsum.tile([DH + 1, QBW], F32,
                                name=f"ob{qb_}_{pr}", tag="o")
                eps = {}

                def emit_ttpv(g):
                    """mask-multiply + PV for kb group (2g, 2g+1) — emitted
                    several k-blocks late so the PE's in-order queue never
                    parks a PV (waiting on exp) in front of a ready QK."""
                    ppg = eps[g].rearrange("p a (b f) -> p a b f", b=2)
                    mh = met_tiles[(qb_, g // 2)]
                    base = mh[:, (2 * g) % 4, :]
                    mdup = bass.AP(
                        tensor=mh.tensor,
                        offset=base.offset,
                        ap=[mh.ap[0], [QBW, 2], [0, 2], [1, QBW]])
                    nc.vector.tensor_tensor(
                        out=ppg, in0=ppg, in1=mdup, op=ALU.mult)
                    for dkb in (2 * g, 2 * g + 1):
                        for o_ps, h, half in ((oa, hA, 0), (ob, hB, 1)):
                            nc.tensor.matmul(
                                o_ps,
                                V[:, dkb, h, :],
                                ppg[:, dkb % 2, half, :],
                                start=(dkb == 0),
                                stop=(dkb == NKB - 1))

                for kb in range(NKB):
                    if met_kb is not None and kb == met_kb:
                        for c in range(4):
                            emit_met(qb_, c)
                    sp = spsum.tile([128, 1024], F32,
                                    name=f"sp{qb_}_{pr}_{kb}", tag="s")
                    nc.tensor.matmul(
                        sp[:, 0:512],
                        KT[0:64, pr, kb * 128:(kb + 1) * 128],
                        QT[0:64, pr, q0:q0 + QBW],
                        start=True, stop=True, tile_position=(0, 0))
                    nc.tensor.matmul(
                        sp[:, 512:1024],
                        KT[64:128, pr, kb * 128:(kb + 1) * 128],
                        QT[64:128, pr, q0:q0 + QBW],
                        start=True, stop=True, tile_position=(64, 0))
                    if kb % 2 == 0:
                        eps[kb // 2] = epool.tile(
                            [128, 2, 1024], BF16,
                            name=f"e{qb_}_{pr}_{kb}", tag="e")
                    nc.scalar.activation(eps[kb // 2][:, kb % 2, :],
                                         sp, AF.Exp)
                    if kb % 2 == 1:
                        pump(rate)
                        if kb >= delay:
                            emit_ttpv((kb - delay) // 2)
                def postproc():
                    stage = stagep.tile([128, 4, 128], F32,
                                        name=f"st{qb_}_{pr}", tag="st")
                    osbA = oevacp.tile([DH + 1, QBW], F32R,
                                       name=f"oeA{qb_}_{pr}", tag="oe")
                    osbB = oevacp.tile([DH + 1, QBW], F32R,
                                       name=f"oeB{qb_}_{pr}", tag="oe")
                    nc.vector.tensor_copy(osbA, oa)
                    nc.vector.tensor_copy(osbB, ob)
                    for osb, h in ((osbA, hA), (osbB, hB)):
                        hcol = (h % 2) * DH
                        tp = scratch.tile([128, 4 * 65], F32R,
                                          name=f"tq{qb_}_{pr}_{h}", tag="sc")
                        for j in range(4):
                            nc.tensor.transpose(
                                tp[:, j * 65:(j + 1) * 65],
                                osb[:, j * 128:(j + 1) * 128],
                                idf32r[0:65, 0:65])
                        tpv = tp.rearrange("p (j c) -> p j c", j=4)
                        rec = rpool.tile([128, 4], F32,
                                         name=f"rc{qb_}_{pr}_{h}", tag="rc")
                        nc.vector.reciprocal(rec, tpv[:, :, 64:65])
                        for j in range(4):
                            nc.vector.tensor_scalar_mul(
                                stage[:, j, hcol:hcol + DH],
                                tpv[:, j, 0:DH],
                                rec[:, j:j + 1])
                    nc.sync.dma_start(
                        out=out_d[q0:q0 + QBW, pr * 128:(pr + 1) * 128]
                        .rearrange("(j p) c -> p j c", p=128),
                        in_=stage)

                # defer the final PV groups and the postproc to the next
                # unit's first pump call: its early QKs slot in ahead so
                # neither the PV tail nor the postproc transposes ever make
                # the ACT exp stream wait at a unit boundary.  The last
                # unit flushes inline (nothing follows to hide behind).
                if flush_tail:
                    for g in range((NKB - delay + 1) // 2, NKB // 2):
                        emit_ttpv(g)
                    postproc()
                else:
                    gs = (NKB - delay + 1) // 2
                    tail = [lambda g=g: emit_ttpv(g)
                            for g in range(gs, NKB // 2)] + [postproc]
                    for i, fn_ in enumerate(tail):
                        work.insert(min(i * defer_stride, len(work)), fn_)

            # ---------------- emission schedule ---------------------------
            # Head (direct emission): X l0+l1 and the np0 weight columns
            # gate the first K/Q chains so unit (0,0)'s exp stream starts
            # ~12us in.  Everything else — remaining W columns, wv, the
            # qb0 mask pipeline, X l2/l3, the other projection chains —
            # drains from the pump inside unit (0,0) (PE-bound window).
            # Mask pipelines for qb1-3 are deferred into units 1-4 where
            # both DMA and ACT have slack, instead of saturating the head.
            emit_xt_load(0, 0)
            wk0 = wpool.tile([128, NDB, 128], F32R, name="wk0", tag="wk0")
            nc.sync.dma_start(
                out=wk0,
                in_=wk_d[:, 0:128].rearrange("(c p) n -> p c n", p=128))
            wq0 = wpool.tile([128, NDB, 128], F32R, name="wq0", tag="wq0")
            nc.sync.dma_start(
                out=wq0,
                in_=wq_d[:, 0:128].rearrange("(c p) n -> p c n", p=128))
            bqt = wpool.tile([128, NPAIR], F32, name="bqt", tag="bqt")
            nc.sync.dma_start(out=bqt,
                              in_=bq_d.rearrange("o (c p) -> (o p) c", p=128))
            bkt = wpool.tile([128, NPAIR], F32, name="bkt", tag="bkt")
            nc.sync.dma_start(out=bkt,
                              in_=bk_d.rearrange("o (c p) -> (o p) c", p=128))
            bv = wpool.tile([1, 512], F32R, name="bv", tag="bv")
            nc.sync.dma_start(out=bv, in_=bv_d[:, :])
            emit_xt_load(0, 1)
            bvb = pers.tile([128, 512], F32, name="bvb", tag="bvb")

            def emit_bvb():
                # broadcast-bv tile: ones[128,1] (x) bv[1,512], one K=1 matmul
                bvp = scratch.tile([128, 512], F32, name="bvp", tag="sc")
                nc.tensor.matmul(bvp, ones_r[0:1, :], bv,
                                 start=True, stop=True)
                nc.vector.tensor_copy(bvb, bvp)

            # PE warm-up while the first X tiles stream in: ~40 dummy
            # transposes ramp the tensor engine to full p-state so the
            # first real chains run at 2.4GHz instead of 0.65/1.2.
            warm = scratch.tile([128, 128], F32R, name="warm", tag="sc")
            for _ in range(40):
                nc.tensor.transpose(warm, idf32r, idf32r)

            emit_xt(0, 0, evac="act")
            emit_qk_chain(wk0, 0, bkt, KT, 0, 0, half=0)
            emit_qk_chain(wq0, 0, bqt, QT, 0, 0, half=0)
            wk1 = wpool.tile([128, NDB, 384], F32R, name="wk1", tag="wk1")
            nc.sync.dma_start(
                out=wk1,
                in_=wk_d[:, 128:512].rearrange("(c p) n -> p c n", p=128))
            wq1 = wpool.tile([128, NDB, 384], F32R, name="wq1", tag="wq1")
            nc.sync.dma_start(
                out=wq1,
                in_=wq_d[:, 128:512].rearrange("(c p) n -> p c n", p=128))
            emit_xt_load(1, 0)
            emit_xt_load(1, 1)

            wv = wpool.tile([128, NDB, 512], F32R, name="wv", tag="wv")

            def emit_wv_load():
                nc.sync.dma_start(
                    out=wv, in_=wv_d.rearrange("(c p) n -> p c n", p=128))

            mload_tiles = {}

            def mask_load_item(qb):
                mload_tiles[qb] = emit_mask_load(qb)

            def mask_exp_item(qb):
                emit_mask_exp(qb, mload_tiles.pop(qb))

            def W(fn, *a):
                work.append(lambda: fn(*a))

            # ---- unit (0,0) backlog: l-chunk-major projection (the
            # rotating xt buffer requires each chunk's 12 consumers emitted
            # before the next chunk's transposes), with DMA items placed so
            # the SP queue feeds each consumer just in time.  The second
            # X^T half and the h1 chains lead: kb2 needs them.
            def Wxt(lb, sh):
                for db in range(NDB):
                    W(emit_xt_db, lb, sh, db, "act")

            def Wch(w_sb, c0, bias_t, dst, np_, lb, half=None):
                W(emit_qk_chain, w_sb, c0, bias_t, dst, np_, lb, half,
                  "act")

            Wxt(0, 1)
            Wch(wk0, 0, bkt, KT, 0, 0, 1)
            Wch(wq0, 0, bqt, QT, 0, 0, 1)
            W(emit_wv_load)
            for pr in (1, 2, 3):
                Wch(wk1, 1, bkt, KT, pr, 0)
            for pr in (1, 2, 3):
                Wch(wq1, 1, bqt, QT, pr, 0)
            W(emit_bvb)
            for kb in range(4):
                W(emit_v_chain, kb)
            W(emit_xt_load, 2, 0)
            W(emit_xt_load, 2, 1)
            Wxt(1, 0)
            Wxt(1, 1)
            # mask pipeline woven through (mload ring depth 2: never more
            # than two loads outstanding before their exp is emitted, or
            # the in-order SP DMA queue head-blocks on an ACT semaphore)
            W(mask_load_item, 0)
            W(mask_load_item, 1)
            Wch(wk0, 0, bkt, KT, 0, 1)
            for pr in (1, 2, 3):
                Wch(wk1, 1, bkt, KT, pr, 1)
            W(mask_exp_item, 0)
            W(mask_load_item, 2)
            W(mask_exp_item, 1)
            W(mask_load_item, 3)
            Wch(wq0, 0, bqt, QT, 0, 1)
            for pr in (1, 2, 3):
                Wch(wq1, 1, bqt, QT, pr, 1)
            for kb in range(4, 8):
                W(emit_v_chain, kb)
            W(emit_xt_load, 3, 0)
            W(emit_xt_load, 3, 1)
            Wxt(2, 0)
            Wxt(2, 1)
            W(mask_exp_item, 2)
            W(mask_load_item, 4)
            W(mask_exp_item, 3)
            W(mask_load_item, 5)
            Wch(wk0, 0, bkt, KT, 0, 2)
            for pr in (1, 2, 3):
                Wch(wk1, 1, bkt, KT, pr, 2)
            W(mask_exp_item, 4)
            W(mask_load_item, 6)
            W(mask_exp_item, 5)
            W(mask_load_item, 7)
            Wch(wq0, 0, bqt, QT, 0, 2)
            for pr in (1, 2, 3):
                Wch(wq1, 1, bqt, QT, pr, 2)
            W(mask_exp_item, 6)
            W(mask_load_item, 8)
            W(mask_exp_item, 7)
            W(mask_load_item, 9)
            for kb in range(8, 12):
                W(emit_v_chain, kb)
            W(mask_exp_item, 8)
            W(mask_load_item, 10)
            W(mask_exp_item, 9)
            W(mask_load_item, 11)
            Wxt(3, 0)
            Wxt(3, 1)
            Wch(wk0, 0, bkt, KT, 0, 3)
            W(mask_exp_item, 10)
            W(mask_exp_item, 11)
            # --- everything below pumps after met0 fires inline (kb 14) ---
            for kb in range(12, 16):
                W(emit_v_chain, kb)
            W(mask_load_item, 12)
            W(mask_load_item, 13)

            # ---- per-unit scheduled items (appended before each unit):
            # K l3 chains land just before their consumer unit; mask
            # pipelines for qb1-3 spread across units 1-4 (ACT slack in the
            # PE-bound early units, DMA slack everywhere after the head);
            # Q l3 chains before the qb3 row.
            sched = {
                1: [(emit_qk_chain, wk1, 1, bkt, KT, 1, 3, 0),
                    (emit_qk_chain, wk1, 1, bkt, KT, 1, 3, 1),
                    (mask_exp_item, 12), (mask_exp_item, 13),
                    (mask_load_item, 14), (mask_load_item, 15)],
                2: [(emit_qk_chain, wk1, 1, bkt, KT, 2, 3, 0),
                    (emit_qk_chain, wk1, 1, bkt, KT, 2, 3, 1),
                    (mask_exp_item, 14), (mask_exp_item, 15)],
                3: [(emit_qk_chain, wk1, 1, bkt, KT, 3, 3, 0),
                    (emit_qk_chain, wk1, 1, bkt, KT, 3, 3, 1)],
                5: [(emit_qk_chain, wq0, 0, bqt, QT, 0, 3, 0)],
                6: [(emit_qk_chain, wq0, 0, bqt, QT, 0, 3, 1)],
                7: [(emit_qk_chain, wq1, 1, bqt, QT, 1, 3, 0)],
                8: [(emit_qk_chain, wq1, 1, bqt, QT, 1, 3, 1)],
                9: [(emit_qk_chain, wq1, 1, bqt, QT, 2, 3, 0)],
                10: [(emit_qk_chain, wq1, 1, bqt, QT, 2, 3, 1)],
                11: [(emit_qk_chain, wq1, 1, bqt, QT, 3, 3, 0)],
                12: [(emit_qk_chain, wq1, 1, bqt, QT, 3, 3, 1)],
            }

            unit = 0
            for qb_ in range(NQB):
                for pr in range(NPAIR):
                    for item in sched.get(unit, ()):
                        W(*item)
                    if unit == 0:
                        emit_attn_pair(0, 0, rate=10, delay=15, met_kb=14,
                                       defer_stride=3)
                    else:
                        emit_attn_pair(qb_, pr,
                                       rate=(6 if unit == 1 else
                                             4 if unit < 4 else 1),
                                       delay=9 if unit < 15 else 5,
                                       met_kb=0 if pr == 0 else None,
                                       flush_tail=(unit == 15))
                    unit += 1
            pump(len(work))

    nc.finalize()
    return nc


def _get_nc():
    if "nc" not in _CACHE:
        _CACHE["nc"] = _build()
    return _CACHE["nc"]


def kernel(embedding, mask, Wq, bq, Wk, bk, Wv, bv):
    from concourse.bass_utils import run_bass_kernel_spmd

    nc = _get_nc()

    embedding = np.asarray(embedding, dtype=np.float32)
    mask = np.asarray(mask, dtype=np.float32)
    in_maps = []
    for c in range(NCORES):
        b = c // 2
        h0 = (c % 2) * HPC
        cs = slice(h0 * DH, (h0 + HPC) * DH)
        in_maps.append({
            "x": np.ascontiguousarray(embedding[b]),
            "mask": np.ascontiguousarray(mask[b, 0]),
            "wq": np.ascontiguousarray(np.asarray(Wq, np.float32)[:, cs]),
            "wk": np.ascontiguousarray(np.asarray(Wk, np.float32)[:, cs]),
            "wv": np.ascontiguousarray(np.asarray(Wv, np.float32)[:, cs]),
            "bq": np.ascontiguousarray(np.asarray(bq, np.float32)[cs]).reshape(1, 512),
            "bk": np.ascontiguousarray(np.asarray(bk, np.float32)[cs]).reshape(1, 512),
            "bv": np.ascontiguousarray(np.asarray(bv, np.float32)[cs]).reshape(1, 512),
        })

    res = run_bass_kernel_spmd(nc, in_maps, core_ids=list(range(NCORES)))

    out = np.empty((B, L, D), dtype=np.float32)
    for c in range(NCORES):
        b = c // 2
        h0 = (c % 2) * HPC
        out[b][:, h0 * DH:(h0 + HPC) * DH] = res.results[c]["out"]
    return out



# revision 56
# speedup vs baseline: 1.0601x; 1.0601x over previous
"""Trainium2 Bass kernel for nn_Attention_3728031613575.

Multi-head attention, B=4 L=2048 D=1024 H=16 (head dim 64), fp32 reference:
    q/k/v = split_heads(x @ W{q,k,v} + b)        [b,h,l,64]
    scores = q k^T + mask * (-1e5)
    out    = softmax(scores) @ v                 -> [b,l,1024]

Sharding (8 cores): core c handles batch b = c//2 and heads (c%2)*8..+8
(batch x head-group data parallel; QKV weights column-sharded by head).
Attention is fully local per core; no collectives.

Per-core algorithm (layouts chosen so softmax lives on the PSUM partition
dim and no probability transposes are ever needed):
  - X^T built once via PE transposes (fp32r), evacuated on the DVE
    (GPSIMD cannot access PSUM, so all PSUM evacuations live on DVE).
  - Q^T/K^T [head-dims, l] and V [l, head-dims] projections in fp32r;
    Q/K biases folded into the DVE PSUM evacuation as a per-partition
    tensor_scalar_add; V bias as a rank-1 matmul term.
  - mask preprocessed once per core:  M_e = exp(-1e5*(m - rowmin(m)))
    (the rowmin bias provides exact max-subtraction for the mask-dominated
    term; the remaining q.k part is range-safe in fp32), stored bf16,
    reloaded transposed through the DMA xbar on the SP queue.
  - per (head-pair, q-block 512, k-block 128):
      S^T[k,q] = K^T.T @ Q^T        (two K=64 matmuls row-tiled on the PE)
      E = exp(S^T)                  (ACT, bf16, from PSUM)
      P~ = E * M_e^T                (DVE, bf16, 2-kb-wide ops)
      O'^T[d,q] += V^T P~ with a ones-column in V producing the softmax
      denominators as row 64 of O'.
  - postproc: PE-transpose O'^T -> [q, 65] (f32r datapath), reciprocal of
    col 64, tensor_scalar normalize, DMA out.

Scheduling: the QKV projection is split into fine-grained work items
(one transpose pair / one 8-matmul projection chain each) drained from a
pump queue between attention k-blocks, so the PE never executes a long
projection blob while the ACT exp stream starves.  The mask pipeline
(DMA in -> DVE rowmin -> ACT exp -> DMA out -> transposed met loads) is
pumped the same way, with every mask block's pipeline placed inside the
projection window where the ACT engine would otherwise idle; transposed
met tiles load just-in-time at each q-block start, queued ahead of the
pumped mask loads.  Each unit's final PV group and postproc defer into
the next unit's first pump slot so unit boundaries never stall the exp
stream; PV groups trail their QK blocks by 9 k-blocks (15 in the
projection-window unit (0,0), whose met tiles land late), with the tail
groups deferred into the next unit, decoupling the ACT exp stream from
PV bursts on the in-order PE queue — this holds the ACT engine at ~98%
occupancy in steady state.  The projection window itself is co-limited
by the serial DMA resource (fp32 mask loads + X/W traffic).
"""

import sys

sys.path.insert(0, "/opt/trn_rl_repo")

import numpy as np

B, L, D, H, DH = 4, 2048, 1024, 16, 64
NCORES = 8
HPC = 8            # heads per core
NPAIR = HPC // 2   # head pairs per core
QBW = 512          # q block width
NQB = L // QBW     # 4 q blocks
NKB = L // 128     # 16 k blocks
NDB = D // 128     # 8 contraction chunks
MASK_C = -100000.0

_CACHE = {}


def _build():
    import concourse.bass as bass
    from concourse import bacc, mybir
    import concourse.tile as tile
    from concourse.masks import make_identity

    F32 = mybir.dt.float32
    F32R = mybir.dt.float32r
    BF16 = mybir.dt.bfloat16
    AF = mybir.ActivationFunctionType
    ALU = mybir.AluOpType
    AX = mybir.AxisListType

    nc = bacc.Bacc(None, target_bir_lowering=False)

    x_d = nc.dram_tensor("x", [L, D], F32R, kind="ExternalInput")
    mask_d = nc.dram_tensor("mask", [L, L], F32, kind="ExternalInput")
    wq_d = nc.dram_tensor("wq", [D, 512], F32R, kind="ExternalInput")
    wk_d = nc.dram_tensor("wk", [D, 512], F32R, kind="ExternalInput")
    wv_d = nc.dram_tensor("wv", [D, 512], F32R, kind="ExternalInput")
    bq_d = nc.dram_tensor("bq", [1, 512], F32, kind="ExternalInput")
    bk_d = nc.dram_tensor("bk", [1, 512], F32, kind="ExternalInput")
    bv_d = nc.dram_tensor("bv", [1, 512], F32R, kind="ExternalInput")
    out_d = nc.dram_tensor("out", [L, 512], F32, kind="ExternalOutput")

    with tile.TileContext(nc) as tc:
        with tc.tile_pool(name="const", bufs=1) as constp, \
             tc.tile_pool(name="persist", bufs=1) as pers, \
             tc.tile_pool(name="dram", bufs=1, space="DRAM") as dramp, \
             tc.tile_pool(name="met", bufs=4) as metp, \
             tc.tile_pool(name="stage", bufs=1) as stagep, \
             tc.tile_pool(name="epool", bufs=3) as epool, \
             tc.tile_pool(name="oevac", bufs=2) as oevacp, \
             tc.tile_pool(name="rpool", bufs=4) as rpool, \
             tc.tile_pool(name="wpool", bufs=1) as wpool, \
             tc.tile_pool(name="xload", bufs=3) as xload, \
             tc.tile_pool(name="xtpool", bufs=1) as xtpool, \
             tc.tile_pool(name="mload", bufs=2) as mload, \
             tc.tile_pool(name="mtmp", bufs=2) as mtmp, \
             tc.tile_pool(name="spsum", bufs=2, space="PSUM") as spsum, \
             tc.tile_pool(name="opsum", bufs=2, space="PSUM") as opsum, \
             tc.tile_pool(name="scratch", bufs=2, space="PSUM") as scratch:

            # ---- constants
            idf32r = constp.tile([128, 128], F32R, name="idf32r", tag="idf32r")
            make_identity(nc, idf32r)
            ones_r = constp.tile([1, 128], F32R, name="ones_r", tag="ones_r")
            nc.vector.memset(ones_r, 1.0)

            # ---- persistent activations
            QT = pers.tile([128, NPAIR, L], F32R, name="QT", tag="QT")
            KT = pers.tile([128, NPAIR, L], F32R, name="KT", tag="KT")
            V = pers.tile([128, NKB, HPC, DH + 1], BF16, name="V", tag="V")
            nc.vector.memset(V[:, :, :, DH], 1.0)

            me_dram = dramp.tile([L, L], BF16, name="me_dram", tag="me_dram")

            # ---- weight / bias loads (DMA queue order matters: X l0 +
            # np0 weight columns gate the first QK chain; masks for qb0
            # follow; everything else pumps behind)

            xts = {}

            def xt_tile(lb):
                if lb not in xts:
                    xts[lb] = xtpool.tile([128, NDB, QBW], F32R,
                                          name="xt%d" % lb, tag="xt")
                return xts[lb]

            # ---------------- mask pipeline (half-width tiles) -----------
            def emit_mask_load(qb):
                mls = []
                for h in range(2):
                    ml = mload.tile([128, L // 2], F32,
                                    name=f"ml{qb}_{h}", tag="ml")
                    nc.sync.dma_start(
                        out=ml,
                        in_=mask_d[qb * 128:(qb + 1) * 128,
                                   h * (L // 2):(h + 1) * (L // 2)])
                    mls.append(ml)
                return mls

            def emit_mask_exp(qb, mls):
                mm = rpool.tile([128, 2], F32, name=f"mm{qb}", tag="rc")
                for h in range(2):
                    nc.vector.tensor_reduce(mm[:, h:h + 1], mls[h],
                                            axis=AX.X, op=ALU.min)
                mbias = rpool.tile([128, 1], F32, name=f"mb{qb}", tag="rc")
                nc.vector.tensor_reduce(mbias, mm, axis=AX.X, op=ALU.min)
                nc.vector.tensor_scalar_mul(mbias, mbias, -MASK_C)
                for h in range(2):
                    me = mtmp.tile([128, L // 2], BF16,
                                   name=f"me{qb}_{h}", tag="me")
                    nc.scalar.activation(me, mls[h], AF.Exp,
                                         bias=mbias, scale=MASK_C)
                    nc.sync.dma_start(
                        out=me_dram[qb * 128:(qb + 1) * 128,
                                    h * (L // 2):(h + 1) * (L // 2)],
                        in_=me)

            met_tiles = {}

            def emit_met(qb_, c):
                """Transposed M_e tiles for (q block, k-block chunk c)."""
                q0 = qb_ * QBW
                mh = metp.tile([128, 4, QBW], BF16,
                               name=f"met{qb_}_{c}", tag="met")
                met_tiles[(qb_, c)] = mh
                for i in range(4):
                    nc.sync.dma_start_transpose(
                        mh[:, i, :],
                        me_dram[q0:q0 + QBW,
                                (4 * c + i) * 128:(4 * c + i + 1) * 128])

            # ---------------- projection work items ---------------------
            xload_tiles = {}

            def emit_xt_load(lb, sh):
                for s in range(2):
                    xl = xload.tile([128, D], F32R,
                                    name=f"xl{lb}_{sh}_{s}", tag="xl")
                    nc.sync.dma_start(
                        out=xl,
                        in_=x_d[lb * 512 + (sh * 2 + s) * 128:
                                lb * 512 + (sh * 2 + s + 1) * 128, :])
                    xload_tiles[(lb, sh, s)] = xl

            def emit_xt_db(lb, sh, db, evac="dve"):
                """Transpose one 128-wide d-chunk of X rows
                [lb*512+sh*256, +256) into xt; fine-grained so the pump can
                hide the evacuation behind the QK stream."""
                xt = xt_tile(lb)
                xls = [xload_tiles[(lb, sh, s)] for s in range(2)]
                tpt = scratch.tile([128, 256], F32R,
                                   name=f"tpd{lb}_{sh}_{db}", tag="sc")
                for s in range(2):
                    nc.tensor.transpose(
                        tpt[:, s * 128:(s + 1) * 128],
                        xls[s][:, db * 128:(db + 1) * 128],
                        idf32r)
                if db == NDB - 1:
                    for s in range(2):
                        del xload_tiles[(lb, sh, s)]
                dst = xt[:, db, sh * 256:(sh + 1) * 256]
                if evac == "act":
                    # ACT is idle during the head; keep the DVE free
                    nc.scalar.activation(dst, tpt, AF.Copy)
                else:
                    nc.vector.tensor_copy(dst, tpt)

            def emit_xt(lb, sh, evac="dve"):
                if (lb, sh, 0) not in xload_tiles:
                    emit_xt_load(lb, sh)
                for db in range(NDB):
                    emit_xt_db(lb, sh, db, evac)

            def emit_qk_chain(w_sb, c0, bias_t, dst, np_, lb, half=None,
                              evac="dve"):
                """One projection chain into dst (KT slice or QT chunk).
                w_sb holds W columns starting at head-col c0*128.  half
                restricts to a 256-wide l-half (for the head chains that
                must not wait on the second X^T half)."""
                l0, lw = (0, 512) if half is None else (half * 256, 256)
                qp = scratch.tile([128, lw],
                                  F32, name=f"qp{np_}_{lb}_{id(w_sb) % 97}",
                                  tag="sc")
                for db in range(NDB):
                    nc.tensor.matmul(
                        qp,
                        w_sb[:, db, (np_ - c0) * 128:(np_ - c0 + 1) * 128],
                        xt_tile(lb)[:, db, l0:l0 + lw],
                        start=(db == 0), stop=(db == NDB - 1))
                dst_sl = dst[:, np_, lb * 512 + l0:lb * 512 + l0 + lw]
                if evac == "act":
                    nc.scalar.activation(dst_sl, qp, AF.Identity,
                                         bias=bias_t[:, np_:np_ + 1])
                else:
                    nc.vector.tensor_scalar_add(dst_sl, qp,
                                                bias_t[:, np_:np_ + 1])

            def emit_v_chain(kb):
                vp = scratch.tile([128, 512], F32, name=f"vp{kb}", tag="sc")
                for db in range(NDB):
                    nc.tensor.matmul(
                        vp,
                        xt_tile(kb // 4)[:, db, (kb % 4) * 128:
                                         (kb % 4 + 1) * 128],
                        wv[:, db, :],
                        start=(db == 0), stop=(db == NDB - 1))
                # bias folded in via the broadcast tile (built once): V+bv
                nc.vector.tensor_tensor(
                    out=V[:, kb, :, 0:DH],
                    in0=vp.rearrange("p (h d) -> p h d", h=HPC),
                    in1=bvb.rearrange("p (h d) -> p h d", h=HPC),
                    op=ALU.add)

            # ---------------- pump ---------------------------------------
            work = []

            def pump(n):
                for _ in range(n):
                    if work:
                        work.pop(0)()

            # ---------------- attention unit ------------------------------
            def emit_attn_pair(qb_, pr, rate=2, delay=9, met_kb=0,
                               flush_tail=False, defer_stride=1):
                hA, hB = 2 * pr, 2 * pr + 1
                q0 = qb_ * QBW
                oa = opsum.tile([DH + 1, QBW], F32,
                                name=f"oa{qb_}_{pr}", tag="o")
                ob = opsum.tile([DH + 1, QBW], F32,
                                name=f"ob{qb_}_{pr}", tag="o")
                eps = {}

                def emit_ttpv(g):
                    """mask-multiply + PV for kb group (2g, 2g+1) — emitted
                    several k-blocks late so the PE's in-order queue never
                    parks a PV (waiting on exp) in front of a ready QK."""
                    ppg = eps[g].rearrange("p a (b f) -> p a b f", b=2)
                    mh = met_tiles[(qb_, g // 2)]
                    base = mh[:, (2 * g) % 4, :]
                    mdup = bass.AP(
                        tensor=mh.tensor,
                        offset=base.offset,
                        ap=[mh.ap[0], [QBW, 2], [0, 2], [1, QBW]])
                    nc.vector.tensor_tensor(
                        out=ppg, in0=ppg, in1=mdup, op=ALU.mult)
                    for dkb in (2 * g, 2 * g + 1):
                        for o_ps, h, half in ((oa, hA, 0), (ob, hB, 1)):
                            nc.tensor.matmul(
                                o_ps,
                                V[:, dkb, h, :],
                                ppg[:, dkb % 2, half, :],
                                start=(dkb == 0),
                                stop=(dkb == NKB - 1))

                for kb in range(NKB):
                    if met_kb is not None and kb == met_kb:
                        for c in range(4):
                            emit_met(qb_, c)
                    sp = spsum.tile([128, 1024], F32,
                                    name=f"sp{qb_}_{pr}_{kb}", tag="s")
                    nc.tensor.matmul(
                        sp[:, 0:512],
                        KT[0:64, pr, kb * 128:(kb + 1) * 128],
                        QT[0:64, pr, q0:q0 + QBW],
                        start=True, stop=True, tile_position=(0, 0))
                    nc.tensor.matmul(
                        sp[:, 512:1024],
                        KT[64:128, pr, kb * 128:(kb + 1) * 128],
                        QT[64:128, pr, q0:q0 + QBW],
                        start=True, stop=True, tile_position=(64, 0))
                    if kb % 2 == 0:
                        eps[kb // 2] = epool.tile(
                            [128, 2, 1024], BF16,
                            name=f"e{qb_}_{pr}_{kb}", tag="e")
                    nc.scalar.activation(eps[kb // 2][:, kb % 2, :],
                                         sp, AF.Exp)
                    if kb % 2 == 1:
                        pump(rate)
                        if kb >= delay:
                            emit_ttpv((kb - delay) // 2)
                def postproc():
                    stage = stagep.tile([128, 4, 128], F32,
                                        name=f"st{qb_}_{pr}", tag="st")
                    osbA = oevacp.tile([DH + 1, QBW], F32R,
                                       name=f"oeA{qb_}_{pr}", tag="oe")
                    osbB = oevacp.tile([DH + 1, QBW], F32R,
                                       name=f"oeB{qb_}_{pr}", tag="oe")
                    nc.vector.tensor_copy(osbA, oa)
                    nc.vector.tensor_copy(osbB, ob)
                    for osb, h in ((osbA, hA), (osbB, hB)):
                        hcol = (h % 2) * DH
                        tp = scratch.tile([128, 4 * 65], F32R,
                                          name=f"tq{qb_}_{pr}_{h}", tag="sc")
                        for j in range(4):
                            nc.tensor.transpose(
                                tp[:, j * 65:(j + 1) * 65],
                                osb[:, j * 128:(j + 1) * 128],
                                idf32r[0:65, 0:65])
                        tpv = tp.rearrange("p (j c) -> p j c", j=4)
                        rec = rpool.tile([128, 4], F32,
                                         name=f"rc{qb_}_{pr}_{h}", tag="rc")
                        nc.vector.reciprocal(rec, tpv[:, :, 64:65])
                        for j in range(4):
                            nc.vector.tensor_scalar_mul(
                                stage[:, j, hcol:hcol + DH],
                                tpv[:, j, 0:DH],
                                rec[:, j:j + 1])
                    nc.sync.dma_start(
                        out=out_d[q0:q0 + QBW, pr * 128:(pr + 1) * 128]
                        .rearrange("(j p) c -> p j c", p=128),
                        in_=stage)

                # defer the final PV groups and the postproc to the next
                # unit's first pump call: its early QKs slot in ahead so
                # neither the PV tail nor the postproc transposes ever make
                # the ACT exp stream wait at a unit boundary.  The last
                # unit flushes inline (nothing follows to hide behind).
                if flush_tail:
                    for g in range((NKB - delay + 1) // 2, NKB // 2):
                        emit_ttpv(g)
                    postproc()
                else:
                    gs = (NKB - delay + 1) // 2
                    tail = [lambda g=g: emit_ttpv(g)
                            for g in range(gs, NKB // 2)] + [postproc]
                    for i, fn_ in enumerate(tail):
                        work.insert(min(i * defer_stride, len(work)), fn_)

            # ---------------- emission schedule ---------------------------
            # Head (direct emission): X l0+l1 and the np0 weight columns
            # gate the first K/Q chains so unit (0,0)'s exp stream starts
            # ~12us in.  Everything else — remaining W columns, wv, the
            # qb0 mask pipeline, X l2/l3, the other projection chains —
            # drains from the pump inside unit (0,0) (PE-bound window).
            # Mask pipelines for qb1-3 are deferred into units 1-4 where
            # both DMA and ACT have slack, instead of saturating the head.
            emit_xt_load(0, 0)
            wk0 = wpool.tile([128, NDB, 128], F32R, name="wk0", tag="wk0")
            nc.sync.dma_start(
                out=wk0,
                in_=wk_d[:, 0:128].rearrange("(c p) n -> p c n", p=128))
            wq0 = wpool.tile([128, NDB, 128], F32R, name="wq0", tag="wq0")
            nc.sync.dma_start(
                out=wq0,
                in_=wq_d[:, 0:128].rearrange("(c p) n -> p c n", p=128))
            bqt = wpool.tile([128, NPAIR], F32, name="bqt", tag="bqt")
            nc.sync.dma_start(out=bqt,
                              in_=bq_d.rearrange("o (c p) -> (o p) c", p=128))
            bkt = wpool.tile([128, NPAIR], F32, name="bkt", tag="bkt")
            nc.sync.dma_start(out=bkt,
                              in_=bk_d.rearrange("o (c p) -> (o p) c", p=128))
            bv = wpool.tile([1, 512], F32R, name="bv", tag="bv")
            nc.sync.dma_start(out=bv, in_=bv_d[:, :])
            emit_xt_load(0, 1)
            bvb = pers.tile([128, 512], F32, name="bvb", tag="bvb")

            def emit_bvb():
                # broadcast-bv tile: ones[128,1] (x) bv[1,512], one K=1 matmul
                bvp = scratch.tile([128, 512], F32, name="bvp", tag="sc")
                nc.tensor.matmul(bvp, ones_r[0:1, :], bv,
                                 start=True, stop=True)
                nc.vector.tensor_copy(bvb, bvp)

            # PE warm-up while the first X tiles stream in: ~40 dummy
            # transposes ramp the tensor engine to full p-state so the
            # first real chains run at 2.4GHz instead of 0.65/1.2.
            warm = scratch.tile([128, 128], F32R, name="warm", tag="sc")
            for _ in range(40):
                nc.tensor.transpose(warm, idf32r, idf32r)

            emit_xt(0, 0, evac="act")
            emit_qk_chain(wk0, 0, bkt, KT, 0, 0, half=0)
            emit_qk_chain(wq0, 0, bqt, QT, 0, 0, half=0)
            wk1 = wpool.tile([128, NDB, 384], F32R, name="wk1", tag="wk1")
            nc.sync.dma_start(
                out=wk1,
                in_=wk_d[:, 128:512].rearrange("(c p) n -> p c n", p=128))
            wq1 = wpool.tile([128, NDB, 384], F32R, name="wq1", tag="wq1")
            nc.sync.dma_start(
                out=wq1,
                in_=wq_d[:, 128:512].rearrange("(c p) n -> p c n", p=128))
            emit_xt_load(1, 0)
            emit_xt_load(1, 1)

            wv = wpool.tile([128, NDB, 512], F32R, name="wv", tag="wv")

            def emit_wv_load():
                nc.sync.dma_start(
                    out=wv, in_=wv_d.rearrange("(c p) n -> p c n", p=128))

            mload_tiles = {}

            def mask_load_item(qb):
                mload_tiles[qb] = emit_mask_load(qb)

            def mask_exp_item(qb):
                emit_mask_exp(qb, mload_tiles.pop(qb))

            def W(fn, *a):
                work.append(lambda: fn(*a))

            # ---- unit (0,0) backlog: l-chunk-major projection (the
            # rotating xt buffer requires each chunk's 12 consumers emitted
            # before the next chunk's transposes), with DMA items placed so
            # the SP queue feeds each consumer just in time.  The second
            # X^T half and the h1 chains lead: kb2 needs them.
            def Wxt(lb, sh):
                for db in range(NDB):
                    W(emit_xt_db, lb, sh, db, "act")

            def Wch(w_sb, c0, bias_t, dst, np_, lb, half=None):
                W(emit_qk_chain, w_sb, c0, bias_t, dst, np_, lb, half,
                  "act")

            Wxt(0, 1)
            Wch(wk0, 0, bkt, KT, 0, 0, 1)
            Wch(wq0, 0, bqt, QT, 0, 0, 1)
            W(emit_wv_load)
            for pr in (1, 2, 3):
                Wch(wk1, 1, bkt, KT, pr, 0)
            for pr in (1, 2, 3):
                Wch(wq1, 1, bqt, QT, pr, 0)
            W(emit_bvb)
            for kb in range(4):
                W(emit_v_chain, kb)
            W(emit_xt_load, 2, 0)
            W(emit_xt_load, 2, 1)
            Wxt(1, 0)
            Wxt(1, 1)
            # mask pipeline woven through (mload ring depth 2: never more
            # than two loads outstanding before their exp is emitted, or
            # the in-order SP DMA queue head-blocks on an ACT semaphore)
            W(mask_load_item, 0)
            W(mask_load_item, 1)
            Wch(wk0, 0, bkt, KT, 0, 1)
            for pr in (1, 2, 3):
                Wch(wk1, 1, bkt, KT, pr, 1)
            W(mask_exp_item, 0)
            W(mask_load_item, 2)
            W(mask_exp_item, 1)
            W(mask_load_item, 3)
            Wch(wq0, 0, bqt, QT, 0, 1)
            for pr in (1, 2, 3):
                Wch(wq1, 1, bqt, QT, pr, 1)
            for kb in range(4, 8):
                W(emit_v_chain, kb)
            W(emit_xt_load, 3, 0)
            W(emit_xt_load, 3, 1)
            Wxt(2, 0)
            Wxt(2, 1)
            W(mask_exp_item, 2)
            W(mask_load_item, 4)
            W(mask_exp_item, 3)
            W(mask_load_item, 5)
            Wch(wk0, 0, bkt, KT, 0, 2)
            for pr in (1, 2, 3):
                Wch(wk1, 1, bkt, KT, pr, 2)
            W(mask_exp_item, 4)
            W(mask_load_item, 6)
            W(mask_exp_item, 5)
            W(mask_load_item, 7)
            Wch(wq0, 0, bqt, QT, 0, 2)
            for pr in (1, 2, 3):
                Wch(wq1, 1, bqt, QT, pr, 2)
            W(mask_exp_item, 6)
            W(mask_load_item, 8)
            W(mask_exp_item, 7)
            W(mask_load_item, 9)
            for kb in range(8, 12):
                W(emit_v_chain, kb)
            W(mask_exp_item, 8)
            W(mask_load_item, 10)
            W(mask_exp_item, 9)
            W(mask_load_item, 11)
            Wxt(3, 0)
            Wxt(3, 1)
            Wch(wk0, 0, bkt, KT, 0, 3)
            W(mask_exp_item, 10)
            W(mask_exp_item, 11)
            # --- everything below pumps after met0 fires inline (kb 14) ---
            for kb in range(12, 16):
                W(emit_v_chain, kb)
            W(mask_load_item, 12)
            W(mask_load_item, 13)

            # ---- per-unit scheduled items (appended before each unit):
            # K l3 chains land just before their consumer unit; mask
            # pipelines for qb1-3 spread across units 1-4 (ACT slack in the
            # PE-bound early units, DMA slack everywhere after the head);
            # Q l3 chains before the qb3 row.
            sched = {
                1: [(emit_qk_chain, wk1, 1, bkt, KT, 1, 3, 0),
                    (emit_qk_chain, wk1, 1, bkt, KT, 1, 3, 1),
                    (mask_exp_item, 12), (mask_exp_item, 13),
                    (mask_load_item, 14), (mask_load_item, 15)],
                2: [(emit_qk_chain, wk1, 1, bkt, KT, 2, 3, 0),
                    (emit_qk_chain, wk1, 1, bkt, KT, 2, 3, 1),
                    (mask_exp_item, 14), (mask_exp_item, 15)],
                3: [(emit_qk_chain, wk1, 1, bkt, KT, 3, 3, 0),
                    (emit_qk_chain, wk1, 1, bkt, KT, 3, 3, 1)],
                5: [(emit_qk_chain, wq0, 0, bqt, QT, 0, 3, 0)],
                6: [(emit_qk_chain, wq0, 0, bqt, QT, 0, 3, 1)],
                7: [(emit_qk_chain, wq1, 1, bqt, QT, 1, 3, 0)],
                8: [(emit_qk_chain, wq1, 1, bqt, QT, 1, 3, 1)],
                9: [(emit_qk_chain, wq1, 1, bqt, QT, 2, 3, 0)],
                10: [(emit_qk_chain, wq1, 1, bqt, QT, 2, 3, 1)],
                11: [(emit_qk_chain, wq1, 1, bqt, QT, 3, 3, 0)],
                12: [(emit_qk_chain, wq1, 1, bqt, QT, 3, 3, 1)],
            }

            unit = 0
            for qb_ in range(NQB):
                for pr in range(NPAIR):
                    for item in sched.get(unit, ()):
                        W(*item)
                    if unit == 0:
                        emit_attn_pair(0, 0, rate=24, delay=15, met_kb=14,
                                       defer_stride=3)
                    else:
                        emit_attn_pair(qb_, pr,
                                       rate=(6 if unit == 1 else
                                             4 if unit < 4 else 1),
                                       delay=9 if unit < 15 else 5,
                                       met_kb=0 if pr == 0 else None,
                                       flush_tail=(unit == 15))
                    unit += 1
            pump(len(work))

    nc.finalize()
    return nc


def _get_nc():
    if "nc" not in _CACHE:
        _CACHE["nc"] = _build()
    return _CACHE["nc"]


def kernel(embedding, mask, Wq, bq, Wk, bk, Wv, bv):
    from concourse.bass_utils import run_bass_kernel_spmd

    nc = _get_nc()

    embedding = np.asarray(embedding, dtype=np.float32)
    mask = np.asarray(mask, dtype=np.float32)
    in_maps = []
    for c in range(NCORES):
        b = c // 2
        h0 = (c % 2) * HPC
        cs = slice(h0 * DH, (h0 + HPC) * DH)
        in_maps.append({
            "x": np.ascontiguousarray(embedding[b]),
            "mask": np.ascontiguousarray(mask[b, 0]),
            "wq": np.ascontiguousarray(np.asarray(Wq, np.float32)[:, cs]),
            "wk": np.ascontiguousarray(np.asarray(Wk, np.float32)[:, cs]),
            "wv": np.ascontiguousarray(np.asarray(Wv, np.float32)[:, cs]),
            "bq": np.ascontiguousarray(np.asarray(bq, np.float32)[cs]).reshape(1, 512),
            "bk": np.ascontiguousarray(np.asarray(bk, np.float32)[cs]).reshape(1, 512),
            "bv": np.ascontiguousarray(np.asarray(bv, np.float32)[cs]).reshape(1, 512),
        })

    res = run_bass_kernel_spmd(nc, in_maps, core_ids=list(range(NCORES)))

    out = np.empty((B, L, D), dtype=np.float32)
    for c in range(NCORES):
        b = c // 2
        h0 = (c % 2) * HPC
        out[b][:, h0 * DH:(h0 + HPC) * DH] = res.results[c]["out"]
    return out

